# revision 15
# baseline (speedup 1.0000x reference)
"""Trainium2 Bass kernel for nn_AttentionModel (greedy pointer-attention decode).

Contract: kernel(**inputs) takes FULL inputs (B=1024), shards batch across 8
NeuronCores (128 items each, SPMD), runs the 199-step greedy decode on-device,
returns full (1024, 199, 200) float32 log_p.

v2 design (all per-step matmuls algebraically folded away; DVE-centric):
  precompute (row-tile loop over (b,n) rows):
    e2 = emb + pref
    [gK | gV | lK] = e2 @ W_node  (PE fp32)
    K  = gK * isqrt(32)  -> bf16 rows, (d,h)-major cols -> DRAM -> resident SBUF
    V  = gV              -> bf16 rows, (d,h)-major cols -> DRAM (streamed)
    lK'= (lK @ W_out^T) * isqrt(256) -> bf16 (d,h)-major -> DRAM (streamed)
    S  = e2 @ W_step[256:] ((d,h)-major cols) -> DRAM f32 (gathered per step)
    fixed2 = mean(e2) @ W_fixed + e2[:,24] @ W_step[:256]  ((d,h)-major)
  per step (no matmuls at all; bf16 products + pairwise halving trees on DVE):
    q = fixed2 + S[prev]
    compat[n,h] = sum_e K[n,e,h]*q[e,h]
    attn = exp(compat+amask)/sum        (no max-sub; bounded by construction)
    glimpse[e,h] = sum_n V[n,e,h]*attn[n,h]   (streamed V chunks)
    logits[n] = sum_c lK'[n,c]*g[c]           (streamed lK' chunks)
    lm = 10*tanh(logits) + amask ; store row (host adds -logsumexp later)
    sel = argmax (DVE max/max_index); amask update; next q gather.
"""
import numpy as np

import concourse.bass as bass
from concourse import bacc
import concourse.tile as tile
from concourse import mybir
from concourse.bass import IndirectOffsetOnAxis
from concourse.bass_utils import run_bass_kernel_spmd

dt = mybir.dt
F32 = dt.float32
BF16 = dt.bfloat16
AX = mybir.AxisListType
OP = mybir.AluOpType
ACTF = mybir.ActivationFunctionType

B, N, D, H = 1024, 200, 256, 8
dd = D // H                     # 32
NCORES = 8
BS = B // NCORES                # 128 items per core
T = N - 1                       # 199 decode steps
START = 24
NEG = -1e9
NC = 20                         # n-chunk size
NCH = N // NC                   # 10 chunks
MC = 10                         # mean-pass chunk
ISD = float(np.float32(1.0 / np.sqrt(32.0)))
ISD256 = 0.0625
ROWT = BS * N // 128            # 200 row-tiles in precompute
DBG = False


def _build():
    nc = bacc.Bacc("TRN2", target_bir_lowering=False, debug=False)

    emb_in = nc.dram_tensor("embeddings", [BS, N, D], F32, kind="ExternalInput").ap()
    pref_in = nc.dram_tensor("pref_embed", [D], F32, kind="ExternalInput").ap()
    wnode_in = nc.dram_tensor("W_node", [D, 3 * D], F32, kind="ExternalInput").ap()
    wfix_in = nc.dram_tensor("W_fixed", [D, D], F32, kind="ExternalInput").ap()
    wstep_in = nc.dram_tensor("W_step", [2 * D, D], F32, kind="ExternalInput").ap()
    wout_in = nc.dram_tensor("W_out", [D, D], F32, kind="ExternalInput").ap()

    out = nc.dram_tensor("log_p", [BS, T * N], F32, kind="ExternalOutput").ap()

    if DBG:
        dbg_fixed2 = nc.dram_tensor("dbg_fixed2", [BS, D], F32, kind="ExternalOutput").ap()
        dbg_q = nc.dram_tensor("dbg_q", [BS, D], F32, kind="ExternalOutput").ap()
        dbg_compat = nc.dram_tensor("dbg_compat", [BS, N * H], F32, kind="ExternalOutput").ap()
        dbg_attn = nc.dram_tensor("dbg_attn", [BS, N * H], F32, kind="ExternalOutput").ap()
        dbg_g = nc.dram_tensor("dbg_g", [BS, D], F32, kind="ExternalOutput").ap()
        dbg_logits = nc.dram_tensor("dbg_logits", [BS, N], F32, kind="ExternalOutput").ap()
        dbg_sel = nc.dram_tensor("dbg_sel", [BS, 1], dt.int32, kind="ExternalOutput").ap()
        dbg_krow = nc.dram_tensor("dbg_krow", [BS, D], F32, kind="ExternalOutput").ap()
        dbg_lrow = nc.dram_tensor("dbg_lrow", [BS, D], F32, kind="ExternalOutput").ap()
        dbg_srow = nc.dram_tensor("dbg_srow", [BS, D], F32, kind="ExternalOutput").ap()
    e2_d = nc.dram_tensor("e2_d", [BS * N, D], F32).ap()
    krow_d = nc.dram_tensor("krow_d", [BS * N, D], F32).ap()
    vrow_d = nc.dram_tensor("vrow_d", [BS * N, D], F32).ap()
    lrow_d = nc.dram_tensor("lrow_d", [BS * N, D], F32).ap()
    s_d = nc.dram_tensor("s_d", [BS * N, D], F32).ap()

    with tile.TileContext(nc) as tc:
        with (
            tc.tile_pool(name="wpool", bufs=1) as wpool,
            tc.tile_pool(name="stream", bufs=4) as stream,
            tc.tile_pool(name="prod", bufs=2) as prodp,
            tc.tile_pool(name="work", bufs=2) as work,
            tc.tile_pool(name="small", bufs=1) as small,
        ):
          with (
            tc.tile_pool(name="wpre", bufs=1) as wpre,
            tc.tile_pool(name="pwork", bufs=1) as pwork,
            tc.tile_pool(name="psA", bufs=2, space="PSUM") as psA,
            tc.tile_pool(name="psB", bufs=2, space="PSUM") as psB,
            tc.tile_pool(name="psT", bufs=2, space="PSUM") as psT,
          ):
            # ---------------- precompute-only weights ----------------
            wn_sb = wpre.tile([128, 2, 3 * D], F32)
            nc.sync.dma_start(wn_sb[:, 0, :], wnode_in[0:128, :])
            nc.sync.dma_start(wn_sb[:, 1, :], wnode_in[128:256, :])
            ws2_sb = wpre.tile([128, 2, D], F32)       # W_step[256:512]
            nc.sync.dma_start(ws2_sb[:, 0, :], wstep_in[256:384, :])
            nc.sync.dma_start(ws2_sb[:, 1, :], wstep_in[384:512, :])
            ws1_sb = wpre.tile([128, 2, D], F32)       # W_step[0:256]
            nc.sync.dma_start(ws1_sb[:, 0, :], wstep_in[0:128, :])
            nc.sync.dma_start(ws1_sb[:, 1, :], wstep_in[128:256, :])
            wf_sb = wpre.tile([128, 2, D], F32)
            nc.sync.dma_start(wf_sb[:, 0, :], wfix_in[0:128, :])
            nc.sync.dma_start(wf_sb[:, 1, :], wfix_in[128:256, :])
            wo_sb = wpre.tile([128, 2, D], F32)        # W_out row-tiles
            nc.sync.dma_start(wo_sb[:, 0, :], wout_in[0:128, :])
            nc.sync.dma_start(wo_sb[:, 1, :], wout_in[128:256, :])

            pref_sb = wpre.tile([128, D], F32)
            nc.sync.dma_start(
                pref_sb[:],
                pref_in.rearrange("(o f) -> o f", o=1).broadcast_to([128, D]),
            )

            ident = wpre.tile([128, 128], F32)
            io_c = wpre.tile([128, 128], dt.int32)
            nc.gpsimd.iota(io_c[:], pattern=[[1, 128]], channel_multiplier=0)
            io_r = wpre.tile([128, 1], dt.int32)
            nc.gpsimd.iota(io_r[:], pattern=[[0, 1]], channel_multiplier=1)
            id_i = wpre.tile([128, 128], dt.int32)
            nc.vector.tensor_tensor(id_i[:], io_c[:], io_r[:].broadcast_to([128, 128]), op=OP.is_equal)
            nc.vector.tensor_copy(ident[:], id_i[:])

            iota_n = wpool.tile([128, N], dt.int32)
            nc.gpsimd.iota(iota_n[:], pattern=[[1, N]], channel_multiplier=0)
            iota_row = wpool.tile([128, 1], dt.int32)   # p*N
            nc.gpsimd.iota(iota_row[:], pattern=[[0, 1]], channel_multiplier=N)

            amask = wpool.tile([128, N], F32)
            nc.vector.memset(amask[:], 0.0)
            nc.vector.memset(amask[:, START:START + 1], NEG)

            sel = wpool.tile([128, 1], dt.int32)
            selF = wpool.tile([128, 1], F32)
            nc.vector.memset(selF[:], float(START))
            nc.vector.tensor_copy(sel[:], selF[:])

            fixed2 = wpool.tile([128, D], F32)

            # W_out^T in SBUF: woT2[:, jt, i] = W_out[i, j]
            woT2 = wpre.tile([128, 2, D], F32)
            for jt in range(2):
                for it in range(2):
                    tp = psT.tile([128, 128], F32, tag="tp")
                    nc.tensor.transpose(tp[:], wo_sb[:, it, jt * 128:(jt + 1) * 128], ident[:])
                    nc.scalar.copy(woT2[:, jt, it * 128:(it + 1) * 128], tp[:])


            # ---------------- precompute row-tile loop ----------------
            emb_rows = emb_in.rearrange("b n c -> (b n) c")

            def dhv(t2):
                # natural (h,e)-major [p, 256] -> [p, e, h] view (reorder cols)
                return t2.rearrange("p (h e) -> p e h", h=H)

            def ehs(t2):
                # contiguous (e,h)-major [p, 256] -> [p, e, h] view (plain split)
                return t2.rearrange("p (e h) -> p e h", h=H)

            def pre_body(rt):
                r0 = rt * 128
                e2 = pwork.tile([128, D], F32, tag="e2")
                nc.sync.dma_start(e2[:], emb_rows[bass.ds(r0, 128), :])
                nc.vector.tensor_tensor(e2[:], e2[:], pref_sb[:], op=OP.add)
                nc.sync.dma_start(e2_d[bass.ds(r0, 128), :], e2[:])
                e2T = pwork.tile([128, 2, 128], F32, tag="e2T")
                for ci in range(2):
                    tp = psT.tile([128, 128], F32, tag="tp")
                    nc.tensor.transpose(tp[:], e2[:, ci * 128:(ci + 1) * 128], ident[:])
                    nc.scalar.copy(e2T[:, ci, :], tp[:])
                # kvl = e2 @ W_node : psum [512] + [256]
                pa = psA.tile([128, 512], F32, tag="pa")
                nc.tensor.matmul(pa[:], e2T[:, 0, :], wn_sb[:, 0, 0:512], start=True, stop=False)
                nc.tensor.matmul(pa[:], e2T[:, 1, :], wn_sb[:, 1, 0:512], start=False, stop=True)
                pb = psB.tile([128, D], F32, tag="pbx")
                nc.tensor.matmul(pb[:], e2T[:, 0, :], wn_sb[:, 0, 512:768], start=True, stop=False)
                nc.tensor.matmul(pb[:], e2T[:, 1, :], wn_sb[:, 1, 512:768], start=False, stop=True)
                # K row (scaled, (d,h)-major) and V row
                krow = pwork.tile([128, D], F32, tag="krow")
                nc.scalar.activation(krow[:], pa[:, 0:256], ACTF.Copy, scale=ISD)
                nc.sync.dma_start(krow_d[bass.ds(r0, 128), :], krow[:])
                vrow = pwork.tile([128, D], F32, tag="krow")
                nc.vector.tensor_copy(vrow[:], pa[:, 256:512])
                nc.sync.dma_start(vrow_d[bass.ds(r0, 128), :], vrow[:])
                # lK' = (lK @ W_out^T) * ISD256, (d,h)-major via rhs view
                lrow = pwork.tile([128, D], F32, tag="lrow")
                nc.scalar.copy(lrow[:], pb[:])
                lrT = pwork.tile([128, 2, 128], F32, tag="lrT")
                for ci in range(2):
                    tp = psT.tile([128, 128], F32, tag="tp")
                    nc.tensor.transpose(tp[:], lrow[:, ci * 128:(ci + 1) * 128], ident[:])
                    nc.scalar.copy(lrT[:, ci, :], tp[:])
                pc = psB.tile([128, D], F32, tag="pbx")
                nc.tensor.matmul(pc[:], lrT[:, 0, :], woT2[:, 0, :], start=True, stop=False)
                nc.tensor.matmul(pc[:], lrT[:, 1, :], woT2[:, 1, :], start=False, stop=True)
                lprow = pwork.tile([128, D], F32, tag="krow")
                nc.scalar.activation(lprow[:], pc[:], ACTF.Copy, scale=ISD256)
                nc.sync.dma_start(lrow_d[bass.ds(r0, 128), :], lprow[:])
                # S row = e2 @ W_step[256:], (d,h)-major
                pdm = psB.tile([128, D], F32, tag="pbx")
                nc.tensor.matmul(pdm[:], e2T[:, 0, :], ws2_sb[:, 0, :], start=True, stop=False)
                nc.tensor.matmul(pdm[:], e2T[:, 1, :], ws2_sb[:, 1, :], start=False, stop=True)
                srow = pwork.tile([128, D], F32, tag="lrow")
                nc.vector.tensor_copy(srow[:], pdm[:])
                nc.sync.dma_start(s_d[bass.ds(r0, 128), :], srow[:])

            tc.For_i_unrolled(0, ROWT, 1, pre_body, max_unroll=2)

            # -------- fixed2 = mean(e2) @ Wf + e2[:,24] @ Ws1, (d,h)-major --------
            macc = wpool.tile([128, D], F32)
            e2_bnc = e2_d.rearrange("(b n) c -> b n c", b=BS)
            for c in range(N // MC):
                ech = stream.tile([128, MC, D], F32, tag="ch")
                nc.sync.dma_start(ech[:], e2_bnc[:, c * MC:(c + 1) * MC, :])
                part = pwork.tile([128, D], F32, tag="e2")
                nc.vector.tensor_reduce(part[:], ech[:].transpose([0, 2, 1]), axis=AX.X, op=OP.add)
                if c == 0:
                    nc.vector.tensor_copy(macc[:], part[:])
                else:
                    nc.vector.tensor_tensor(macc[:], macc[:], part[:], op=OP.add)
            nc.vector.tensor_scalar(macc[:], macc[:], 1.0 / N, None, op0=OP.mult)
            first_sb = wpool.tile([128, D], F32)
            nc.sync.dma_start(first_sb[:], e2_bnc[:, START, :])

            fT = pwork.tile([128, 2, 128], F32, tag="e2T")
            mT = pwork.tile([128, 2, 128], F32, tag="lrT")
            for ci in range(2):
                tp = psT.tile([128, 128], F32, tag="tp")
                nc.tensor.transpose(tp[:], macc[:, ci * 128:(ci + 1) * 128], ident[:])
                nc.scalar.copy(mT[:, ci, :], tp[:])
                tp2 = psT.tile([128, 128], F32, tag="tp")
                nc.tensor.transpose(tp2[:], first_sb[:, ci * 128:(ci + 1) * 128], ident[:])
                nc.scalar.copy(fT[:, ci, :], tp2[:])
            pf = psA.tile([128, 512], F32, tag="pa")
            nc.tensor.matmul(pf[:, 0:256], mT[:, 0, :], wf_sb[:, 0, :], start=True, stop=False)
            nc.tensor.matmul(pf[:, 0:256], mT[:, 1, :], wf_sb[:, 1, :], start=False, stop=False)
            nc.tensor.matmul(pf[:, 0:256], fT[:, 0, :], ws1_sb[:, 0, :], start=False, stop=False)
            nc.tensor.matmul(pf[:, 0:256], fT[:, 1, :], ws1_sb[:, 1, :], start=False, stop=True)
            nc.vector.tensor_copy(fixed2[:], pf[:, 0:256])

            krow_bn = krow_d.rearrange("(b n) c -> b n c", b=BS)

          # ---------------- decode steps (fp32; DVE/POOL split) ----------------
          if True:
            compat = wpool.tile([128, N, H], F32)
            attn = wpool.tile([128, N, H], F32)
            logits = wpool.tile([128, N], F32)
            gacc = wpool.tile([128, D], F32)
            qf = wpool.tile([128, D], F32)

            vrow_bn = vrow_d.rearrange("(b n) c -> b n c", b=BS)
            lrow_bn = lrow_d.rearrange("(b n) c -> b n c", b=BS)
            NDV = 10         # chunks 0..NDV-1 on DVE, rest on POOL

            def eng(c):
                return nc.vector if c < NDV else nc.gpsimd

            def step_body(t):
                # q = fixed2 + S[prev]
                offs = small.tile([128, 1], dt.int32, tag="offs")
                nc.vector.tensor_tensor(offs[:], iota_row[:], sel[:], op=OP.add)
                srow = small.tile([128, D], F32, tag="sgath")
                nc.gpsimd.indirect_dma_start(
                    out=srow[:], out_offset=None,
                    in_=s_d, in_offset=IndirectOffsetOnAxis(ap=offs[:], axis=0))
                nc.vector.tensor_tensor(qf[:], fixed2[:], srow[:], op=OP.add)
                qbb = qf[:].rearrange("p (n c) -> p n c", n=1).broadcast_to([128, NC, D])

                # ---- compat[n,h] = sum_e K[n,h,e]*q[h,e] ----
                for c in range(NCH):
                    n0 = c * NC
                    kch = stream.tile([128, NC, D], F32, tag="ch")
                    nc.sync.dma_start(kch[:], krow_bn[:, n0:n0 + NC, :])
                    pr = prodp.tile([128, NC, D], F32, tag="pr")
                    eng(c).tensor_tensor(pr[:], kch[:], qbb, op=OP.mult)
                    nc.vector.tensor_reduce(
                        compat[:, n0:n0 + NC, :],
                        pr[:].rearrange("p n (h e) -> p n h e", h=H),
                        axis=AX.X, op=OP.add)
                # softmax over n per h (max-sub for safety)
                nc.vector.tensor_tensor(
                    compat[:], compat[:],
                    amask[:].rearrange("p (n o) -> p n o", o=1).broadcast_to([128, N, H]),
                    op=OP.add)
                cmax = small.tile([128, H], F32, tag="cmax")
                nc.vector.tensor_reduce(cmax[:], compat[:].transpose([0, 2, 1]), axis=AX.X, op=OP.max)
                nc.vector.tensor_tensor(
                    compat[:], compat[:],
                    cmax[:].rearrange("p (o h) -> p o h", o=1).broadcast_to([128, N, H]),
                    op=OP.subtract)
                nc.scalar.activation(attn[:], compat[:], ACTF.Exp)
                ssum = small.tile([128, H], F32, tag="ssum")
                nc.vector.tensor_reduce(ssum[:], attn[:].transpose([0, 2, 1]), axis=AX.X, op=OP.add)
                rh = small.tile([128, H], F32, tag="rh")
                nc.vector.reciprocal(rh[:], ssum[:])
                nc.vector.tensor_tensor(
                    attn[:], attn[:],
                    rh[:].rearrange("p (o h) -> p o h", o=1).broadcast_to([128, N, H]),
                    op=OP.mult)

                # ---- glimpse[h,e] = sum_n V[n,h,e]*attn[n,h] ----
                for c in range(NCH):
                    n0 = c * NC
                    vch = stream.tile([128, NC, D], F32, tag="ch")
                    nc.sync.dma_start(vch[:], vrow_bn[:, n0:n0 + NC, :])
                    pr2 = prodp.tile([128, NC, D], F32, tag="pr")
                    eng(c).tensor_tensor(
                        pr2[:].rearrange("p n (h e) -> p n h e", h=H),
                        vch[:].rearrange("p n (h e) -> p n h e", h=H),
                        attn[:, n0:n0 + NC, :].rearrange("p n (h o) -> p n h o", o=1)
                            .broadcast_to([128, NC, H, dd]),
                        op=OP.mult)
                    # in-place halving tree over n (contiguous reads)
                    nc.vector.tensor_tensor(pr2[:, 0:4], pr2[:, 0:4], pr2[:, 16:20], op=OP.add)
                    w = 16
                    while w > 1:
                        w //= 2
                        nc.vector.tensor_tensor(pr2[:, 0:w], pr2[:, 0:w], pr2[:, w:2 * w], op=OP.add)
                    if c == 0:
                        nc.vector.tensor_copy(gacc[:], pr2[:, 0, :])
                    else:
                        nc.vector.tensor_tensor(gacc[:], gacc[:], pr2[:, 0, :], op=OP.add)
                gbb = gacc[:].rearrange("p (n c) -> p n c", n=1).broadcast_to([128, NC, D])

                # ---- logits[n] = sum_c lK'[n,c]*g[c] ----
                for c in range(NCH):
                    n0 = c * NC
                    lch = stream.tile([128, NC, D], F32, tag="ch")
                    nc.sync.dma_start(lch[:], lrow_bn[:, n0:n0 + NC, :])
                    pr3 = prodp.tile([128, NC, D], F32, tag="pr")
                    eng(c).tensor_tensor(pr3[:], lch[:], gbb, op=OP.mult)
                    nc.vector.tensor_reduce(
                        logits[:, n0:n0 + NC], pr3[:], axis=AX.X, op=OP.add)

                # ---- tanh clip, mask, store (host does -logsumexp) ----
                tnh = work.tile([128, N], F32, tag="tnh")
                nc.scalar.activation(tnh[:], logits[:], ACTF.Tanh)
                lm = work.tile([128, N], F32, tag="lm")
                nc.vector.tensor_scalar(lm[:], tnh[:], 10.0, None, op0=OP.mult)
                nc.vector.tensor_tensor(lm[:], lm[:], amask[:], op=OP.add)
                nc.sync.dma_start(out[:, bass.ds(t * N, N)], lm[:])

                # ---- argmax + state update ----
                mx8 = small.tile([128, 8], F32, tag="mx8")
                nc.vector.max(mx8[:], lm[:])
                ix8 = small.tile([128, 8], dt.uint32, tag="ix8")
                nc.vector.max_index(ix8[:], mx8[:], lm[:])
                nc.vector.tensor_copy(sel[:], ix8[:, 0:1])
                ohi = small.tile([128, N], dt.int32, tag="ohi")
                nc.vector.tensor_tensor(ohi[:], iota_n[:], sel[:].broadcast_to([128, N]), op=OP.is_equal)
                ohf = small.tile([128, N], F32, tag="ohf")
                nc.vector.tensor_copy(ohf[:], ohi[:])
                nc.vector.scalar_tensor_tensor(
                    amask[:], ohf[:], NEG, amask[:], op0=OP.mult, op1=OP.add)

            if DBG:
                step_body(0)
                nc.sync.dma_start(dbg_fixed2[:, :], fixed2[:])
                nc.sync.dma_start(dbg_q[:, :], qf[:])
                nc.sync.dma_start(dbg_compat[:, :], compat[:].rearrange("p n h -> p (n h)"))
                nc.vector.tensor_copy(compat[:], attn[:])
                nc.sync.dma_start(dbg_attn[:, :], compat[:].rearrange("p n h -> p (n h)"))
                nc.vector.tensor_copy(qf[:], gacc[:])
                nc.sync.dma_start(dbg_g[:, :], qf[:])
                nc.sync.dma_start(dbg_logits[:, :], logits[:])
                nc.sync.dma_start(dbg_sel[:, :], sel[:])
                kch0 = stream.tile([128, NC, D], F32, tag="ch")
                nc.sync.dma_start(kch0[:], krow_bn[:, 20:40, :])
                nc.vector.tensor_copy(qf[:], kch0[:, 10, :])
                nc.sync.dma_start(dbg_krow[:, :], qf[:])
                lch0 = stream.tile([128, NC, D], F32, tag="ch")
                nc.sync.dma_start(lch0[:], lrow_bn[:, 20:40, :])
                nc.vector.tensor_copy(qf[:], lch0[:, 10, :])
                nc.sync.dma_start(dbg_lrow[:, :], qf[:])
                sr0 = small.tile([128, D], F32, tag="sgath")
                nc.sync.dma_start(sr0[:], s_d.rearrange("(b n) c -> b n c", b=BS)[:, 24, :])
                nc.sync.dma_start(dbg_srow[:, :], sr0[:])
            else:
                tc.For_i_unrolled(0, T, 1, step_body, max_unroll=2)

    nc.compile()
    return nc


_CACHE = {}


def kernel(**inputs) -> np.ndarray:
    if "nc" not in _CACHE:
        _CACHE["nc"] = _build()
    nc = _CACHE["nc"]

    emb = np.ascontiguousarray(np.asarray(inputs["embeddings"], np.float32))
    shared = {
        "pref_embed": np.asarray(inputs["pref_embed"], np.float32),
        "W_node": np.asarray(inputs["W_node"], np.float32),
        "W_fixed": np.asarray(inputs["W_fixed"], np.float32),
        "W_step": np.asarray(inputs["W_step"], np.float32),
        "W_out": np.asarray(inputs["W_out"], np.float32),
    }
    in_maps = []
    for i in range(NCORES):
        m = {"embeddings": emb[i * BS:(i + 1) * BS]}
        m.update(shared)
        in_maps.append(m)

    res = run_bass_kernel_spmd(nc, in_maps, list(range(NCORES)))
    outs = [res.results[i]["log_p"].reshape(BS, T, N) for i in range(NCORES)]
    lm = np.concatenate(outs, axis=0)  # (B, T, N): 10*tanh + mask, pre-normalization
    # host-side log_softmax normalization (exact, float64)
    x = lm.astype(np.float64)
    xf = np.where(x > -1e8, x, -np.inf)
    mx = xf.max(axis=2, keepdims=True)
    lse = mx + np.log(np.exp(xf - mx).sum(axis=2, keepdims=True))
    return (x - lse).astype(np.float32)


if __name__ == "__main__":
    z = np.load("inputs.npz")
    inp = {k: z[k] for k in z.files}
    o = kernel(**inp)
    print("kernel output", o.shape, o.dtype)
    np.save("kernel_out.npy", o)


# revision 20
# speedup vs baseline: 1.0193x; 1.0193x over previous
"""Trainium2 Bass kernel for nn_AttentionModel (greedy pointer-attention decode).

Contract: kernel(**inputs) takes FULL inputs (B=1024), shards batch across 8
NeuronCores (128 items each, SPMD), runs the 199-step greedy decode on-device,
returns full (1024, 199, 200) float32 log_p.

v2 design (all per-step matmuls algebraically folded away; DVE-centric):
  precompute (row-tile loop over (b,n) rows):
    e2 = emb + pref
    [gK | gV | lK] = e2 @ W_node  (PE fp32)
    K  = gK * isqrt(32)  -> bf16 rows, (d,h)-major cols -> DRAM -> resident SBUF
    V  = gV              -> bf16 rows, (d,h)-major cols -> DRAM (streamed)
    lK'= (lK @ W_out^T) * isqrt(256) -> bf16 (d,h)-major -> DRAM (streamed)
    S  = e2 @ W_step[256:] ((d,h)-major cols) -> DRAM f32 (gathered per step)
    fixed2 = mean(e2) @ W_fixed + e2[:,24] @ W_step[:256]  ((d,h)-major)
  per step (no matmuls at all; bf16 products + pairwise halving trees on DVE):
    q = fixed2 + S[prev]
    compat[n,h] = sum_e K[n,e,h]*q[e,h]
    attn = exp(compat+amask)/sum        (no max-sub; bounded by construction)
    glimpse[e,h] = sum_n V[n,e,h]*attn[n,h]   (streamed V chunks)
    logits[n] = sum_c lK'[n,c]*g[c]           (streamed lK' chunks)
    lm = 10*tanh(logits) + amask ; store row (host adds -logsumexp later)
    sel = argmax (DVE max/max_index); amask update; next q gather.
"""
import numpy as np

import concourse.bass as bass
from concourse import bacc
import concourse.tile as tile
from concourse import mybir
from concourse.bass import IndirectOffsetOnAxis
from concourse.bass_utils import run_bass_kernel_spmd

dt = mybir.dt
F32 = dt.float32
BF16 = dt.bfloat16
AX = mybir.AxisListType
OP = mybir.AluOpType
ACTF = mybir.ActivationFunctionType

B, N, D, H = 1024, 200, 256, 8
dd = D // H                     # 32
NCORES = 8
BS = B // NCORES                # 128 items per core
T = N - 1                       # 199 decode steps
START = 24
NEG = -1e9
NC = 20                         # n-chunk size
NCH = N // NC                   # 10 chunks
MC = 10                         # mean-pass chunk
ISD = float(np.float32(1.0 / np.sqrt(32.0)))
ISD256 = 0.0625
ROWT = BS * N // 128            # 200 row-tiles in precompute
KR = 56                         # K columns resident in SBUF (rest streamed)
DBG = False


def _build():
    nc = bacc.Bacc("TRN2", target_bir_lowering=False, debug=False)

    emb_in = nc.dram_tensor("embeddings", [BS, N, D], F32, kind="ExternalInput").ap()
    pref_in = nc.dram_tensor("pref_embed", [D], F32, kind="ExternalInput").ap()
    wnode_in = nc.dram_tensor("W_node", [D, 3 * D], F32, kind="ExternalInput").ap()
    wfix_in = nc.dram_tensor("W_fixed", [D, D], F32, kind="ExternalInput").ap()
    wstep_in = nc.dram_tensor("W_step", [2 * D, D], F32, kind="ExternalInput").ap()
    wout_in = nc.dram_tensor("W_out", [D, D], F32, kind="ExternalInput").ap()

    out = nc.dram_tensor("log_p", [BS, T * N], F32, kind="ExternalOutput").ap()

    if DBG:
        dbg_fixed2 = nc.dram_tensor("dbg_fixed2", [BS, D], F32, kind="ExternalOutput").ap()
        dbg_q = nc.dram_tensor("dbg_q", [BS, D], F32, kind="ExternalOutput").ap()
        dbg_compat = nc.dram_tensor("dbg_compat", [BS, N * H], F32, kind="ExternalOutput").ap()
        dbg_attn = nc.dram_tensor("dbg_attn", [BS, N * H], F32, kind="ExternalOutput").ap()
        dbg_g = nc.dram_tensor("dbg_g", [BS, D], F32, kind="ExternalOutput").ap()
        dbg_logits = nc.dram_tensor("dbg_logits", [BS, N], F32, kind="ExternalOutput").ap()
        dbg_sel = nc.dram_tensor("dbg_sel", [BS, 1], dt.int32, kind="ExternalOutput").ap()
        dbg_krow = nc.dram_tensor("dbg_krow", [BS, D], F32, kind="ExternalOutput").ap()
        dbg_lrow = nc.dram_tensor("dbg_lrow", [BS, D], F32, kind="ExternalOutput").ap()
        dbg_srow = nc.dram_tensor("dbg_srow", [BS, D], F32, kind="ExternalOutput").ap()
    e2_d = nc.dram_tensor("e2_d", [BS * N, D], F32).ap()
    krow_d = nc.dram_tensor("krow_d", [BS * N, D], F32).ap()
    vrow_d = nc.dram_tensor("vrow_d", [BS * N, D], F32).ap()
    lrow_d = nc.dram_tensor("lrow_d", [BS * N, D], F32).ap()
    s_d = nc.dram_tensor("s_d", [BS * N, D], F32).ap()

    with tile.TileContext(nc) as tc:
        with (
            tc.tile_pool(name="wpool", bufs=1) as wpool,
            tc.tile_pool(name="stream", bufs=3) as stream,
            tc.tile_pool(name="prod", bufs=2) as prodp,
            tc.tile_pool(name="work", bufs=2) as work,
            tc.tile_pool(name="small", bufs=1) as small,
        ):
          with (
            tc.tile_pool(name="wpre", bufs=1) as wpre,
            tc.tile_pool(name="pwork", bufs=1) as pwork,
            tc.tile_pool(name="psA", bufs=2, space="PSUM") as psA,
            tc.tile_pool(name="psB", bufs=2, space="PSUM") as psB,
            tc.tile_pool(name="psT", bufs=2, space="PSUM") as psT,
          ):
            # ---------------- precompute-only weights ----------------
            wn_sb = wpre.tile([128, 2, 3 * D], F32)
            nc.sync.dma_start(wn_sb[:, 0, :], wnode_in[0:128, :])
            nc.sync.dma_start(wn_sb[:, 1, :], wnode_in[128:256, :])
            ws2_sb = wpre.tile([128, 2, D], F32)       # W_step[256:512]
            nc.sync.dma_start(ws2_sb[:, 0, :], wstep_in[256:384, :])
            nc.sync.dma_start(ws2_sb[:, 1, :], wstep_in[384:512, :])
            ws1_sb = wpre.tile([128, 2, D], F32)       # W_step[0:256]
            nc.sync.dma_start(ws1_sb[:, 0, :], wstep_in[0:128, :])
            nc.sync.dma_start(ws1_sb[:, 1, :], wstep_in[128:256, :])
            wf_sb = wpre.tile([128, 2, D], F32)
            nc.sync.dma_start(wf_sb[:, 0, :], wfix_in[0:128, :])
            nc.sync.dma_start(wf_sb[:, 1, :], wfix_in[128:256, :])
            wo_sb = wpre.tile([128, 2, D], F32)        # W_out row-tiles
            nc.sync.dma_start(wo_sb[:, 0, :], wout_in[0:128, :])
            nc.sync.dma_start(wo_sb[:, 1, :], wout_in[128:256, :])

            pref_sb = wpre.tile([128, D], F32)
            nc.sync.dma_start(
                pref_sb[:],
                pref_in.rearrange("(o f) -> o f", o=1).broadcast_to([128, D]),
            )

            ident = wpre.tile([128, 128], F32)
            io_c = wpre.tile([128, 128], dt.int32)
            nc.gpsimd.iota(io_c[:], pattern=[[1, 128]], channel_multiplier=0)
            io_r = wpre.tile([128, 1], dt.int32)
            nc.gpsimd.iota(io_r[:], pattern=[[0, 1]], channel_multiplier=1)
            id_i = wpre.tile([128, 128], dt.int32)
            nc.vector.tensor_tensor(id_i[:], io_c[:], io_r[:].broadcast_to([128, 128]), op=OP.is_equal)
            nc.vector.tensor_copy(ident[:], id_i[:])

            iota_n = wpool.tile([128, N], dt.int32)
            nc.gpsimd.iota(iota_n[:], pattern=[[1, N]], channel_multiplier=0)
            iota_row = wpool.tile([128, 1], dt.int32)   # p*N
            nc.gpsimd.iota(iota_row[:], pattern=[[0, 1]], channel_multiplier=N)

            amask = wpool.tile([128, N], F32)
            nc.vector.memset(amask[:], 0.0)
            nc.vector.memset(amask[:, START:START + 1], NEG)

            sel = wpool.tile([128, 1], dt.int32)
            selF = wpool.tile([128, 1], F32)
            nc.vector.memset(selF[:], float(START))
            nc.vector.tensor_copy(sel[:], selF[:])

            fixed2 = wpool.tile([128, D], F32)

            # W_out^T in SBUF: woT2[:, jt, i] = W_out[i, j]
            woT2 = wpre.tile([128, 2, D], F32)
            for jt in range(2):
                for it in range(2):
                    tp = psT.tile([128, 128], F32, tag="tp")
                    nc.tensor.transpose(tp[:], wo_sb[:, it, jt * 128:(jt + 1) * 128], ident[:])
                    nc.scalar.copy(woT2[:, jt, it * 128:(it + 1) * 128], tp[:])


            # ---------------- precompute row-tile loop ----------------
            emb_rows = emb_in.rearrange("b n c -> (b n) c")

            def dhv(t2):
                # natural (h,e)-major [p, 256] -> [p, e, h] view (reorder cols)
                return t2.rearrange("p (h e) -> p e h", h=H)

            def ehs(t2):
                # contiguous (e,h)-major [p, 256] -> [p, e, h] view (plain split)
                return t2.rearrange("p (e h) -> p e h", h=H)

            def pre_body(rt):
                r0 = rt * 128
                e2 = pwork.tile([128, D], F32, tag="e2")
                nc.sync.dma_start(e2[:], emb_rows[bass.ds(r0, 128), :])
                nc.vector.tensor_tensor(e2[:], e2[:], pref_sb[:], op=OP.add)
                nc.sync.dma_start(e2_d[bass.ds(r0, 128), :], e2[:])
                e2T = pwork.tile([128, 2, 128], F32, tag="e2T")
                for ci in range(2):
                    tp = psT.tile([128, 128], F32, tag="tp")
                    nc.tensor.transpose(tp[:], e2[:, ci * 128:(ci + 1) * 128], ident[:])
                    nc.scalar.copy(e2T[:, ci, :], tp[:])
                # kvl = e2 @ W_node : psum [512] + [256]
                pa = psA.tile([128, 512], F32, tag="pa")
                nc.tensor.matmul(pa[:], e2T[:, 0, :], wn_sb[:, 0, 0:512], start=True, stop=False)
                nc.tensor.matmul(pa[:], e2T[:, 1, :], wn_sb[:, 1, 0:512], start=False, stop=True)
                pb = psB.tile([128, D], F32, tag="pbx")
                nc.tensor.matmul(pb[:], e2T[:, 0, :], wn_sb[:, 0, 512:768], start=True, stop=False)
                nc.tensor.matmul(pb[:], e2T[:, 1, :], wn_sb[:, 1, 512:768], start=False, stop=True)
                # K row (scaled, (d,h)-major) and V row
                krow = pwork.tile([128, D], F32, tag="krow")
                nc.scalar.activation(krow[:], pa[:, 0:256], ACTF.Copy, scale=ISD)
                nc.sync.dma_start(krow_d[bass.ds(r0, 128), :], krow[:])
                vrow = pwork.tile([128, D], F32, tag="krow")
                nc.vector.tensor_copy(vrow[:], pa[:, 256:512])
                nc.sync.dma_start(vrow_d[bass.ds(r0, 128), :], vrow[:])
                # lK' = (lK @ W_out^T) * ISD256, (d,h)-major via rhs view
                lrow = pwork.tile([128, D], F32, tag="lrow")
                nc.scalar.copy(lrow[:], pb[:])
                lrT = pwork.tile([128, 2, 128], F32, tag="lrT")
                for ci in range(2):
                    tp = psT.tile([128, 128], F32, tag="tp")
                    nc.tensor.transpose(tp[:], lrow[:, ci * 128:(ci + 1) * 128], ident[:])
                    nc.scalar.copy(lrT[:, ci, :], tp[:])
                pc = psB.tile([128, D], F32, tag="pbx")
                nc.tensor.matmul(pc[:], lrT[:, 0, :], woT2[:, 0, :], start=True, stop=False)
                nc.tensor.matmul(pc[:], lrT[:, 1, :], woT2[:, 1, :], start=False, stop=True)
                lprow = pwork.tile([128, D], F32, tag="krow")
                nc.scalar.activation(lprow[:], pc[:], ACTF.Copy, scale=ISD256)
                nc.sync.dma_start(lrow_d[bass.ds(r0, 128), :], lprow[:])
                # S row = e2 @ W_step[256:], (d,h)-major
                pdm = psB.tile([128, D], F32, tag="pbx")
                nc.tensor.matmul(pdm[:], e2T[:, 0, :], ws2_sb[:, 0, :], start=True, stop=False)
                nc.tensor.matmul(pdm[:], e2T[:, 1, :], ws2_sb[:, 1, :], start=False, stop=True)
                srow = pwork.tile([128, D], F32, tag="lrow")
                nc.vector.tensor_copy(srow[:], pdm[:])
                nc.sync.dma_start(s_d[bass.ds(r0, 128), :], srow[:])

            tc.For_i_unrolled(0, ROWT, 1, pre_body, max_unroll=2)

            # -------- fixed2 = mean(e2) @ Wf + e2[:,24] @ Ws1, (d,h)-major --------
            macc = wpool.tile([128, D], F32)
            e2_bnc = e2_d.rearrange("(b n) c -> b n c", b=BS)
            for c in range(N // MC):
                ech = stream.tile([128, MC, D], F32, tag="ch")
                nc.sync.dma_start(ech[:], e2_bnc[:, c * MC:(c + 1) * MC, :])
                part = pwork.tile([128, D], F32, tag="e2")
                nc.vector.tensor_reduce(part[:], ech[:].transpose([0, 2, 1]), axis=AX.X, op=OP.add)
                if c == 0:
                    nc.vector.tensor_copy(macc[:], part[:])
                else:
                    nc.vector.tensor_tensor(macc[:], macc[:], part[:], op=OP.add)
            nc.vector.tensor_scalar(macc[:], macc[:], 1.0 / N, None, op0=OP.mult)
            first_sb = wpool.tile([128, D], F32)
            nc.sync.dma_start(first_sb[:], e2_bnc[:, START, :])

            fT = pwork.tile([128, 2, 128], F32, tag="e2T")
            mT = pwork.tile([128, 2, 128], F32, tag="lrT")
            for ci in range(2):
                tp = psT.tile([128, 128], F32, tag="tp")
                nc.tensor.transpose(tp[:], macc[:, ci * 128:(ci + 1) * 128], ident[:])
                nc.scalar.copy(mT[:, ci, :], tp[:])
                tp2 = psT.tile([128, 128], F32, tag="tp")
                nc.tensor.transpose(tp2[:], first_sb[:, ci * 128:(ci + 1) * 128], ident[:])
                nc.scalar.copy(fT[:, ci, :], tp2[:])
            pf = psA.tile([128, 512], F32, tag="pa")
            nc.tensor.matmul(pf[:, 0:256], mT[:, 0, :], wf_sb[:, 0, :], start=True, stop=False)
            nc.tensor.matmul(pf[:, 0:256], mT[:, 1, :], wf_sb[:, 1, :], start=False, stop=False)
            nc.tensor.matmul(pf[:, 0:256], fT[:, 0, :], ws1_sb[:, 0, :], start=False, stop=False)
            nc.tensor.matmul(pf[:, 0:256], fT[:, 1, :], ws1_sb[:, 1, :], start=False, stop=True)
            nc.vector.tensor_copy(fixed2[:], pf[:, 0:256])

            krow_bn = krow_d.rearrange("(b n) c -> b n c", b=BS)

          # ---------------- decode steps (fp32; DVE/POOL split) ----------------
          if True:
            K_res = wpool.tile([128, KR, D], F32)
            nc.sync.dma_start(K_res[:], krow_bn[:, 0:KR, :])
            compat = wpool.tile([128, N, H], F32)
            attn = wpool.tile([128, N, H], F32)
            logits = wpool.tile([128, N], F32)
            gacc = wpool.tile([128, D], F32)
            qf = wpool.tile([128, D], F32)

            vrow_bn = vrow_d.rearrange("(b n) c -> b n c", b=BS)
            lrow_bn = lrow_d.rearrange("(b n) c -> b n c", b=BS)
            NDV = 5          # chunks 0..NDV-1 on DVE, rest on POOL

            def eng(c):
                return nc.vector if c < NDV else nc.gpsimd

            def step_body(t):
                # q = fixed2 + S[prev]
                offs = small.tile([128, 1], dt.int32, tag="offs")
                nc.vector.tensor_tensor(offs[:], iota_row[:], sel[:], op=OP.add)
                srow = small.tile([128, D], F32, tag="sgath")
                nc.gpsimd.indirect_dma_start(
                    out=srow[:], out_offset=None,
                    in_=s_d, in_offset=IndirectOffsetOnAxis(ap=offs[:], axis=0))
                nc.vector.tensor_tensor(qf[:], fixed2[:], srow[:], op=OP.add)
                qbb = qf[:].rearrange("p (n c) -> p n c", n=1).broadcast_to([128, NC, D])

                # ---- compat[n,h] = sum_e K[n,h,e]*q[h,e] ----
                for c in range(NCH):
                    n0 = c * NC
                    if n0 + NC <= KR:
                        kch = K_res[:, n0:n0 + NC, :]
                    else:
                        kt = stream.tile([128, NC, D], F32, tag="ch")
                        nc.sync.dma_start(kt[:], krow_bn[:, n0:n0 + NC, :])
                        kch = kt[:]
                    pr = prodp.tile([128, NC, D], F32, tag="pr")
                    eng(c).tensor_tensor(pr[:], kch, qbb, op=OP.mult)
                    nc.vector.tensor_reduce(
                        compat[:, n0:n0 + NC, :],
                        pr[:].rearrange("p n (h e) -> p n h e", h=H),
                        axis=AX.X, op=OP.add)
                # softmax over n per h (max-sub for safety)
                nc.vector.tensor_tensor(
                    compat[:], compat[:],
                    amask[:].rearrange("p (n o) -> p n o", o=1).broadcast_to([128, N, H]),
                    op=OP.add)
                cmax = small.tile([128, H], F32, tag="cmax")
                nc.vector.tensor_reduce(cmax[:], compat[:].transpose([0, 2, 1]), axis=AX.X, op=OP.max)
                nc.vector.tensor_tensor(
                    compat[:], compat[:],
                    cmax[:].rearrange("p (o h) -> p o h", o=1).broadcast_to([128, N, H]),
                    op=OP.subtract)
                nc.scalar.activation(attn[:], compat[:], ACTF.Exp)
                ssum = small.tile([128, H], F32, tag="ssum")
                nc.vector.tensor_reduce(ssum[:], attn[:].transpose([0, 2, 1]), axis=AX.X, op=OP.add)
                rh = small.tile([128, H], F32, tag="rh")
                nc.vector.reciprocal(rh[:], ssum[:])
                nc.vector.tensor_tensor(
                    attn[:], attn[:],
                    rh[:].rearrange("p (o h) -> p o h", o=1).broadcast_to([128, N, H]),
                    op=OP.mult)

                # ---- glimpse[h,e] = sum_n V[n,h,e]*attn[n,h] ----
                for c in range(NCH):
                    n0 = c * NC
                    vch = stream.tile([128, NC, D], F32, tag="ch")
                    nc.sync.dma_start(vch[:], vrow_bn[:, n0:n0 + NC, :])
                    pr2 = prodp.tile([128, NC, D], F32, tag="pr")
                    eng(c).tensor_tensor(
                        pr2[:].rearrange("p n (h e) -> p n h e", h=H),
                        vch[:].rearrange("p n (h e) -> p n h e", h=H),
                        attn[:, n0:n0 + NC, :].rearrange("p n (h o) -> p n h o", o=1)
                            .broadcast_to([128, NC, H, dd]),
                        op=OP.mult)
                    # in-place halving tree over n (contiguous reads)
                    nc.vector.tensor_tensor(pr2[:, 0:4], pr2[:, 0:4], pr2[:, 16:20], op=OP.add)
                    w = 16
                    while w > 1:
                        w //= 2
                        nc.vector.tensor_tensor(pr2[:, 0:w], pr2[:, 0:w], pr2[:, w:2 * w], op=OP.add)
                    if c == 0:
                        nc.vector.tensor_copy(gacc[:], pr2[:, 0, :])
                    else:
                        nc.vector.tensor_tensor(gacc[:], gacc[:], pr2[:, 0, :], op=OP.add)
                gbb = gacc[:].rearrange("p (n c) -> p n c", n=1).broadcast_to([128, NC, D])

                # ---- logits[n] = sum_c lK'[n,c]*g[c] ----
                for c in range(NCH):
                    n0 = c * NC
                    lch = stream.tile([128, NC, D], F32, tag="ch")
                    nc.sync.dma_start(lch[:], lrow_bn[:, n0:n0 + NC, :])
                    pr3 = prodp.tile([128, NC, D], F32, tag="pr")
                    eng(c).tensor_tensor(pr3[:], lch[:], gbb, op=OP.mult)
                    nc.vector.tensor_reduce(
                        logits[:, n0:n0 + NC], pr3[:], axis=AX.X, op=OP.add)

                # ---- tanh clip, mask, store (host does -logsumexp) ----
                tnh = work.tile([128, N], F32, tag="tnh")
                nc.scalar.activation(tnh[:], logits[:], ACTF.Tanh)
                lm = work.tile([128, N], F32, tag="lm")
                nc.vector.tensor_scalar(lm[:], tnh[:], 10.0, None, op0=OP.mult)
                nc.vector.tensor_tensor(lm[:], lm[:], amask[:], op=OP.add)
                nc.sync.dma_start(out[:, bass.ds(t * N, N)], lm[:])

                # ---- argmax + state update ----
                mx8 = small.tile([128, 8], F32, tag="mx8")
                nc.vector.max(mx8[:], lm[:])
                ix8 = small.tile([128, 8], dt.uint32, tag="ix8")
                nc.vector.max_index(ix8[:], mx8[:], lm[:])
                nc.vector.tensor_copy(sel[:], ix8[:, 0:1])
                ohi = small.tile([128, N], dt.int32, tag="ohi")
                nc.vector.tensor_tensor(ohi[:], iota_n[:], sel[:].broadcast_to([128, N]), op=OP.is_equal)
                ohf = small.tile([128, N], F32, tag="ohf")
                nc.vector.tensor_copy(ohf[:], ohi[:])
                nc.vector.scalar_tensor_tensor(
                    amask[:], ohf[:], NEG, amask[:], op0=OP.mult, op1=OP.add)

            if DBG:
                step_body(0)
                nc.sync.dma_start(dbg_fixed2[:, :], fixed2[:])
                nc.sync.dma_start(dbg_q[:, :], qf[:])
                nc.sync.dma_start(dbg_compat[:, :], compat[:].rearrange("p n h -> p (n h)"))
                nc.vector.tensor_copy(compat[:], attn[:])
                nc.sync.dma_start(dbg_attn[:, :], compat[:].rearrange("p n h -> p (n h)"))
                nc.vector.tensor_copy(qf[:], gacc[:])
                nc.sync.dma_start(dbg_g[:, :], qf[:])
                nc.sync.dma_start(dbg_logits[:, :], logits[:])
                nc.sync.dma_start(dbg_sel[:, :], sel[:])
                kch0 = stream.tile([128, NC, D], F32, tag="ch")
                nc.sync.dma_start(kch0[:], krow_bn[:, 20:40, :])
                nc.vector.tensor_copy(qf[:], kch0[:, 10, :])
                nc.sync.dma_start(dbg_krow[:, :], qf[:])
                lch0 = stream.tile([128, NC, D], F32, tag="ch")
                nc.sync.dma_start(lch0[:], lrow_bn[:, 20:40, :])
                nc.vector.tensor_copy(qf[:], lch0[:, 10, :])
                nc.sync.dma_start(dbg_lrow[:, :], qf[:])
                sr0 = small.tile([128, D], F32, tag="sgath")
                nc.sync.dma_start(sr0[:], s_d.rearrange("(b n) c -> b n c", b=BS)[:, 24, :])
                nc.sync.dma_start(dbg_srow[:, :], sr0[:])
            else:
                tc.For_i_unrolled(0, T, 1, step_body, max_unroll=2)

    nc.compile()
    return nc


_CACHE = {}


def kernel(**inputs) -> np.ndarray:
    if "nc" not in _CACHE:
        _CACHE["nc"] = _build()
    nc = _CACHE["nc"]

    emb = np.ascontiguousarray(np.asarray(inputs["embeddings"], np.float32))
    shared = {
        "pref_embed": np.asarray(inputs["pref_embed"], np.float32),
        "W_node": np.asarray(inputs["W_node"], np.float32),
        "W_fixed": np.asarray(inputs["W_fixed"], np.float32),
        "W_step": np.asarray(inputs["W_step"], np.float32),
        "W_out": np.asarray(inputs["W_out"], np.float32),
    }
    in_maps = []
    for i in range(NCORES):
        m = {"embeddings": emb[i * BS:(i + 1) * BS]}
        m.update(shared)
        in_maps.append(m)

    res = run_bass_kernel_spmd(nc, in_maps, list(range(NCORES)))
    outs = [res.results[i]["log_p"].reshape(BS, T, N) for i in range(NCORES)]
    lm = np.concatenate(outs, axis=0)  # (B, T, N): 10*tanh + mask, pre-normalization
    # host-side log_softmax normalization (exact, float64)
    x = lm.astype(np.float64)
    xf = np.where(x > -1e8, x, -np.inf)
    mx = xf.max(axis=2, keepdims=True)
    lse = mx + np.log(np.exp(xf - mx).sum(axis=2, keepdims=True))
    return (x - lse).astype(np.float32)


if __name__ == "__main__":
    z = np.load("inputs.npz")
    inp = {k: z[k] for k in z.files}
    o = kernel(**inp)
    print("kernel output", o.shape, o.dtype)
    np.save("kernel_out.npy", o)


# revision 22
# speedup vs baseline: 1.0347x; 1.0151x over previous
"""Trainium2 Bass kernel for nn_AttentionModel (greedy pointer-attention decode).

Contract: kernel(**inputs) takes FULL inputs (B=1024), shards batch across 8
NeuronCores (128 items each, SPMD), runs the 199-step greedy decode on-device,
returns full (1024, 199, 200) float32 log_p.

v2 design (all per-step matmuls algebraically folded away; DVE-centric):
  precompute (row-tile loop over (b,n) rows):
    e2 = emb + pref
    [gK | gV | lK] = e2 @ W_node  (PE fp32)
    K  = gK * isqrt(32)  -> bf16 rows, (d,h)-major cols -> DRAM -> resident SBUF
    V  = gV              -> bf16 rows, (d,h)-major cols -> DRAM (streamed)
    lK'= (lK @ W_out^T) * isqrt(256) -> bf16 (d,h)-major -> DRAM (streamed)
    S  = e2 @ W_step[256:] ((d,h)-major cols) -> DRAM f32 (gathered per step)
    fixed2 = mean(e2) @ W_fixed + e2[:,24] @ W_step[:256]  ((d,h)-major)
  per step (no matmuls at all; bf16 products + pairwise halving trees on DVE):
    q = fixed2 + S[prev]
    compat[n,h] = sum_e K[n,e,h]*q[e,h]
    attn = exp(compat+amask)/sum        (no max-sub; bounded by construction)
    glimpse[e,h] = sum_n V[n,e,h]*attn[n,h]   (streamed V chunks)
    logits[n] = sum_c lK'[n,c]*g[c]           (streamed lK' chunks)
    lm = 10*tanh(logits) + amask ; store row (host adds -logsumexp later)
    sel = argmax (DVE max/max_index); amask update; next q gather.
"""
import numpy as np

import concourse.bass as bass
from concourse import bacc
import concourse.tile as tile
from concourse import mybir
from concourse.bass import IndirectOffsetOnAxis
from concourse.bass_utils import run_bass_kernel_spmd

dt = mybir.dt
F32 = dt.float32
BF16 = dt.bfloat16
AX = mybir.AxisListType
OP = mybir.AluOpType
ACTF = mybir.ActivationFunctionType

B, N, D, H = 1024, 200, 256, 8
dd = D // H                     # 32
NCORES = 8
BS = B // NCORES                # 128 items per core
T = N - 1                       # 199 decode steps
START = 24
NEG = -1e9
NC = 20                         # n-chunk size
NCH = N // NC                   # 10 chunks
MC = 10                         # mean-pass chunk
ISD = float(np.float32(1.0 / np.sqrt(32.0)))
ISD256 = 0.0625
ROWT = BS * N // 128            # 200 row-tiles in precompute
KR = 56                         # K columns resident in SBUF (rest streamed)
DBG = False


def _build():
    nc = bacc.Bacc("TRN2", target_bir_lowering=False, debug=False)

    emb_in = nc.dram_tensor("embeddings", [BS, N, D], F32, kind="ExternalInput").ap()
    pref_in = nc.dram_tensor("pref_embed", [D], F32, kind="ExternalInput").ap()
    wnode_in = nc.dram_tensor("W_node", [D, 3 * D], F32, kind="ExternalInput").ap()
    wfix_in = nc.dram_tensor("W_fixed", [D, D], F32, kind="ExternalInput").ap()
    wstep_in = nc.dram_tensor("W_step", [2 * D, D], F32, kind="ExternalInput").ap()
    wout_in = nc.dram_tensor("W_out", [D, D], F32, kind="ExternalInput").ap()

    out = nc.dram_tensor("log_p", [BS, T * N], F32, kind="ExternalOutput").ap()

    if DBG:
        dbg_fixed2 = nc.dram_tensor("dbg_fixed2", [BS, D], F32, kind="ExternalOutput").ap()
        dbg_q = nc.dram_tensor("dbg_q", [BS, D], F32, kind="ExternalOutput").ap()
        dbg_compat = nc.dram_tensor("dbg_compat", [BS, N * H], F32, kind="ExternalOutput").ap()
        dbg_attn = nc.dram_tensor("dbg_attn", [BS, N * H], F32, kind="ExternalOutput").ap()
        dbg_g = nc.dram_tensor("dbg_g", [BS, D], F32, kind="ExternalOutput").ap()
        dbg_logits = nc.dram_tensor("dbg_logits", [BS, N], F32, kind="ExternalOutput").ap()
        dbg_sel = nc.dram_tensor("dbg_sel", [BS, 1], dt.int32, kind="ExternalOutput").ap()
        dbg_krow = nc.dram_tensor("dbg_krow", [BS, D], F32, kind="ExternalOutput").ap()
        dbg_lrow = nc.dram_tensor("dbg_lrow", [BS, D], F32, kind="ExternalOutput").ap()
        dbg_srow = nc.dram_tensor("dbg_srow", [BS, D], F32, kind="ExternalOutput").ap()
    e2_d = nc.dram_tensor("e2_d", [BS * N, D], F32).ap()
    krow_d = nc.dram_tensor("krow_d", [BS * N, D], F32).ap()
    vrow_d = nc.dram_tensor("vrow_d", [BS * N, D], F32).ap()
    lrow_d = nc.dram_tensor("lrow_d", [BS * N, D], F32).ap()
    s_d = nc.dram_tensor("s_d", [BS * N, D], F32).ap()

    with tile.TileContext(nc) as tc:
        with (
            tc.tile_pool(name="wpool", bufs=1) as wpool,
            tc.tile_pool(name="stream", bufs=5) as stream,
            tc.tile_pool(name="prod", bufs=2) as prodp,
            tc.tile_pool(name="work", bufs=2) as work,
            tc.tile_pool(name="small", bufs=1) as small,
        ):
          with (
            tc.tile_pool(name="wpre", bufs=1) as wpre,
            tc.tile_pool(name="pwork", bufs=1) as pwork,
            tc.tile_pool(name="psA", bufs=2, space="PSUM") as psA,
            tc.tile_pool(name="psB", bufs=2, space="PSUM") as psB,
            tc.tile_pool(name="psT", bufs=2, space="PSUM") as psT,
          ):
            # ---------------- precompute-only weights ----------------
            wn_sb = wpre.tile([128, 2, 3 * D], F32)
            nc.sync.dma_start(wn_sb[:, 0, :], wnode_in[0:128, :])
            nc.sync.dma_start(wn_sb[:, 1, :], wnode_in[128:256, :])
            ws2_sb = wpre.tile([128, 2, D], F32)       # W_step[256:512]
            nc.sync.dma_start(ws2_sb[:, 0, :], wstep_in[256:384, :])
            nc.sync.dma_start(ws2_sb[:, 1, :], wstep_in[384:512, :])
            ws1_sb = wpre.tile([128, 2, D], F32)       # W_step[0:256]
            nc.sync.dma_start(ws1_sb[:, 0, :], wstep_in[0:128, :])
            nc.sync.dma_start(ws1_sb[:, 1, :], wstep_in[128:256, :])
            wf_sb = wpre.tile([128, 2, D], F32)
            nc.sync.dma_start(wf_sb[:, 0, :], wfix_in[0:128, :])
            nc.sync.dma_start(wf_sb[:, 1, :], wfix_in[128:256, :])
            wo_sb = wpre.tile([128, 2, D], F32)        # W_out row-tiles
            nc.sync.dma_start(wo_sb[:, 0, :], wout_in[0:128, :])
            nc.sync.dma_start(wo_sb[:, 1, :], wout_in[128:256, :])

            pref_sb = wpre.tile([128, D], F32)
            nc.sync.dma_start(
                pref_sb[:],
                pref_in.rearrange("(o f) -> o f", o=1).broadcast_to([128, D]),
            )

            ident = wpre.tile([128, 128], F32)
            io_c = wpre.tile([128, 128], dt.int32)
            nc.gpsimd.iota(io_c[:], pattern=[[1, 128]], channel_multiplier=0)
            io_r = wpre.tile([128, 1], dt.int32)
            nc.gpsimd.iota(io_r[:], pattern=[[0, 1]], channel_multiplier=1)
            id_i = wpre.tile([128, 128], dt.int32)
            nc.vector.tensor_tensor(id_i[:], io_c[:], io_r[:].broadcast_to([128, 128]), op=OP.is_equal)
            nc.vector.tensor_copy(ident[:], id_i[:])

            iota_n = wpool.tile([128, N], dt.int32)
            nc.gpsimd.iota(iota_n[:], pattern=[[1, N]], channel_multiplier=0)
            iota_row = wpool.tile([128, 1], dt.int32)   # p*N
            nc.gpsimd.iota(iota_row[:], pattern=[[0, 1]], channel_multiplier=N)

            amask = wpool.tile([128, N], F32)
            nc.vector.memset(amask[:], 0.0)
            nc.vector.memset(amask[:, START:START + 1], NEG)

            sel = wpool.tile([128, 1], dt.int32)
            selF = wpool.tile([128, 1], F32)
            nc.vector.memset(selF[:], float(START))
            nc.vector.tensor_copy(sel[:], selF[:])

            fixed2 = wpool.tile([128, D], F32)

            # W_out^T in SBUF: woT2[:, jt, i] = W_out[i, j]
            woT2 = wpre.tile([128, 2, D], F32)
            for jt in range(2):
                for it in range(2):
                    tp = psT.tile([128, 128], F32, tag="tp")
                    nc.tensor.transpose(tp[:], wo_sb[:, it, jt * 128:(jt + 1) * 128], ident[:])
                    nc.scalar.copy(woT2[:, jt, it * 128:(it + 1) * 128], tp[:])


            # ---------------- precompute row-tile loop ----------------
            emb_rows = emb_in.rearrange("b n c -> (b n) c")

            def dhv(t2):
                # natural (h,e)-major [p, 256] -> [p, e, h] view (reorder cols)
                return t2.rearrange("p (h e) -> p e h", h=H)

            def ehs(t2):
                # contiguous (e,h)-major [p, 256] -> [p, e, h] view (plain split)
                return t2.rearrange("p (e h) -> p e h", h=H)

            def pre_body(rt):
                r0 = rt * 128
                e2 = pwork.tile([128, D], F32, tag="e2")
                nc.sync.dma_start(e2[:], emb_rows[bass.ds(r0, 128), :])
                nc.vector.tensor_tensor(e2[:], e2[:], pref_sb[:], op=OP.add)
                nc.sync.dma_start(e2_d[bass.ds(r0, 128), :], e2[:])
                e2T = pwork.tile([128, 2, 128], F32, tag="e2T")
                for ci in range(2):
                    tp = psT.tile([128, 128], F32, tag="tp")
                    nc.tensor.transpose(tp[:], e2[:, ci * 128:(ci + 1) * 128], ident[:])
                    nc.scalar.copy(e2T[:, ci, :], tp[:])
                # kvl = e2 @ W_node : psum [512] + [256]
                pa = psA.tile([128, 512], F32, tag="pa")
                nc.tensor.matmul(pa[:], e2T[:, 0, :], wn_sb[:, 0, 0:512], start=True, stop=False)
                nc.tensor.matmul(pa[:], e2T[:, 1, :], wn_sb[:, 1, 0:512], start=False, stop=True)
                pb = psB.tile([128, D], F32, tag="pbx")
                nc.tensor.matmul(pb[:], e2T[:, 0, :], wn_sb[:, 0, 512:768], start=True, stop=False)
                nc.tensor.matmul(pb[:], e2T[:, 1, :], wn_sb[:, 1, 512:768], start=False, stop=True)
                # K row (scaled, (d,h)-major) and V row
                krow = pwork.tile([128, D], F32, tag="krow")
                nc.scalar.activation(krow[:], pa[:, 0:256], ACTF.Copy, scale=ISD)
                nc.sync.dma_start(krow_d[bass.ds(r0, 128), :], krow[:])
                vrow = pwork.tile([128, D], F32, tag="krow")
                nc.vector.tensor_copy(vrow[:], pa[:, 256:512])
                nc.sync.dma_start(vrow_d[bass.ds(r0, 128), :], vrow[:])
                # lK' = (lK @ W_out^T) * ISD256, (d,h)-major via rhs view
                lrow = pwork.tile([128, D], F32, tag="lrow")
                nc.scalar.copy(lrow[:], pb[:])
                lrT = pwork.tile([128, 2, 128], F32, tag="lrT")
                for ci in range(2):
                    tp = psT.tile([128, 128], F32, tag="tp")
                    nc.tensor.transpose(tp[:], lrow[:, ci * 128:(ci + 1) * 128], ident[:])
                    nc.scalar.copy(lrT[:, ci, :], tp[:])
                pc = psB.tile([128, D], F32, tag="pbx")
                nc.tensor.matmul(pc[:], lrT[:, 0, :], woT2[:, 0, :], start=True, stop=False)
                nc.tensor.matmul(pc[:], lrT[:, 1, :], woT2[:, 1, :], start=False, stop=True)
                lprow = pwork.tile([128, D], F32, tag="krow")
                nc.scalar.activation(lprow[:], pc[:], ACTF.Copy, scale=ISD256)
                nc.sync.dma_start(lrow_d[bass.ds(r0, 128), :], lprow[:])
                # S row = e2 @ W_step[256:], (d,h)-major
                pdm = psB.tile([128, D], F32, tag="pbx")
                nc.tensor.matmul(pdm[:], e2T[:, 0, :], ws2_sb[:, 0, :], start=True, stop=False)
                nc.tensor.matmul(pdm[:], e2T[:, 1, :], ws2_sb[:, 1, :], start=False, stop=True)
                srow = pwork.tile([128, D], F32, tag="lrow")
                nc.vector.tensor_copy(srow[:], pdm[:])
                nc.sync.dma_start(s_d[bass.ds(r0, 128), :], srow[:])

            tc.For_i_unrolled(0, ROWT, 1, pre_body, max_unroll=2)

            # -------- fixed2 = mean(e2) @ Wf + e2[:,24] @ Ws1, (d,h)-major --------
            macc = wpool.tile([128, D], F32)
            e2_bnc = e2_d.rearrange("(b n) c -> b n c", b=BS)
            for c in range(N // MC):
                ech = stream.tile([128, MC, D], F32, tag="ch")
                nc.sync.dma_start(ech[:], e2_bnc[:, c * MC:(c + 1) * MC, :])
                part = pwork.tile([128, D], F32, tag="e2")
                nc.vector.tensor_reduce(part[:], ech[:].transpose([0, 2, 1]), axis=AX.X, op=OP.add)
                if c == 0:
                    nc.vector.tensor_copy(macc[:], part[:])
                else:
                    nc.vector.tensor_tensor(macc[:], macc[:], part[:], op=OP.add)
            nc.vector.tensor_scalar(macc[:], macc[:], 1.0 / N, None, op0=OP.mult)
            first_sb = wpool.tile([128, D], F32)
            nc.sync.dma_start(first_sb[:], e2_bnc[:, START, :])

            fT = pwork.tile([128, 2, 128], F32, tag="e2T")
            mT = pwork.tile([128, 2, 128], F32, tag="lrT")
            for ci in range(2):
                tp = psT.tile([128, 128], F32, tag="tp")
                nc.tensor.transpose(tp[:], macc[:, ci * 128:(ci + 1) * 128], ident[:])
                nc.scalar.copy(mT[:, ci, :], tp[:])
                tp2 = psT.tile([128, 128], F32, tag="tp")
                nc.tensor.transpose(tp2[:], first_sb[:, ci * 128:(ci + 1) * 128], ident[:])
                nc.scalar.copy(fT[:, ci, :], tp2[:])
            pf = psA.tile([128, 512], F32, tag="pa")
            nc.tensor.matmul(pf[:, 0:256], mT[:, 0, :], wf_sb[:, 0, :], start=True, stop=False)
            nc.tensor.matmul(pf[:, 0:256], mT[:, 1, :], wf_sb[:, 1, :], start=False, stop=False)
            nc.tensor.matmul(pf[:, 0:256], fT[:, 0, :], ws1_sb[:, 0, :], start=False, stop=False)
            nc.tensor.matmul(pf[:, 0:256], fT[:, 1, :], ws1_sb[:, 1, :], start=False, stop=True)
            nc.vector.tensor_copy(fixed2[:], pf[:, 0:256])

            krow_bn = krow_d.rearrange("(b n) c -> b n c", b=BS)

          # ---------------- decode steps (fp32; DVE/POOL split) ----------------
          if True:
            compat = wpool.tile([128, N, H], F32)
            attn = wpool.tile([128, N, H], F32)
            logits = wpool.tile([128, N], F32)
            gacc = wpool.tile([128, D], F32)
            qf = wpool.tile([128, D], F32)

            vrow_bn = vrow_d.rearrange("(b n) c -> b n c", b=BS)
            lrow_bn = lrow_d.rearrange("(b n) c -> b n c", b=BS)
            NDV = 5          # chunks 0..NDV-1 on DVE, rest on POOL

            def eng(c):
                return nc.vector if c < NDV else nc.gpsimd

            def step_body(t):
                # q = fixed2 + S[prev]
                offs = small.tile([128, 1], dt.int32, tag="offs")
                nc.vector.tensor_tensor(offs[:], iota_row[:], sel[:], op=OP.add)
                srow = small.tile([128, D], F32, tag="sgath")
                nc.gpsimd.indirect_dma_start(
                    out=srow[:], out_offset=None,
                    in_=s_d, in_offset=IndirectOffsetOnAxis(ap=offs[:], axis=0))
                nc.vector.tensor_tensor(qf[:], fixed2[:], srow[:], op=OP.add)
                qbb = qf[:].rearrange("p (n c) -> p n c", n=1).broadcast_to([128, NC, D])

                # ---- compat[n,h] = sum_e K[n,h,e]*q[h,e] ----
                for c in range(NCH):
                    n0 = c * NC
                    kt = stream.tile([128, NC, D], F32, tag="ch")
                    nc.sync.dma_start(kt[:], krow_bn[:, n0:n0 + NC, :])
                    kch = kt[:]
                    pr = prodp.tile([128, NC, D], F32, tag="pr")
                    eng(c).tensor_tensor(pr[:], kch, qbb, op=OP.mult)
                    nc.vector.tensor_reduce(
                        compat[:, n0:n0 + NC, :],
                        pr[:].rearrange("p n (h e) -> p n h e", h=H),
                        axis=AX.X, op=OP.add)
                # softmax over n per h (max-sub for safety)
                nc.vector.tensor_tensor(
                    compat[:], compat[:],
                    amask[:].rearrange("p (n o) -> p n o", o=1).broadcast_to([128, N, H]),
                    op=OP.add)
                cmax = small.tile([128, H], F32, tag="cmax")
                nc.vector.tensor_reduce(cmax[:], compat[:].transpose([0, 2, 1]), axis=AX.X, op=OP.max)
                nc.vector.tensor_tensor(
                    compat[:], compat[:],
                    cmax[:].rearrange("p (o h) -> p o h", o=1).broadcast_to([128, N, H]),
                    op=OP.subtract)
                nc.scalar.activation(attn[:], compat[:], ACTF.Exp)
                ssum = small.tile([128, H], F32, tag="ssum")
                nc.vector.tensor_reduce(ssum[:], attn[:].transpose([0, 2, 1]), axis=AX.X, op=OP.add)
                rh = small.tile([128, H], F32, tag="rh")
                nc.vector.reciprocal(rh[:], ssum[:])
                nc.vector.tensor_tensor(
                    attn[:], attn[:],
                    rh[:].rearrange("p (o h) -> p o h", o=1).broadcast_to([128, N, H]),
                    op=OP.mult)

                # ---- glimpse[h,e] = sum_n V[n,h,e]*attn[n,h] ----
                for c in range(NCH):
                    n0 = c * NC
                    vch = stream.tile([128, NC, D], F32, tag="ch")
                    nc.sync.dma_start(vch[:], vrow_bn[:, n0:n0 + NC, :])
                    pr2 = prodp.tile([128, NC, D], F32, tag="pr")
                    eng(c).tensor_tensor(
                        pr2[:].rearrange("p n (h e) -> p n h e", h=H),
                        vch[:].rearrange("p n (h e) -> p n h e", h=H),
                        attn[:, n0:n0 + NC, :].rearrange("p n (h o) -> p n h o", o=1)
                            .broadcast_to([128, NC, H, dd]),
                        op=OP.mult)
                    # in-place halving tree over n (contiguous reads)
                    nc.vector.tensor_tensor(pr2[:, 0:4], pr2[:, 0:4], pr2[:, 16:20], op=OP.add)
                    w = 16
                    while w > 1:
                        w //= 2
                        nc.vector.tensor_tensor(pr2[:, 0:w], pr2[:, 0:w], pr2[:, w:2 * w], op=OP.add)
                    if c == 0:
                        nc.vector.tensor_copy(gacc[:], pr2[:, 0, :])
                    else:
                        nc.vector.tensor_tensor(gacc[:], gacc[:], pr2[:, 0, :], op=OP.add)
                gbb = gacc[:].rearrange("p (n c) -> p n c", n=1).broadcast_to([128, NC, D])

                # ---- logits[n] = sum_c lK'[n,c]*g[c] ----
                for c in range(NCH):
                    n0 = c * NC
                    lch = stream.tile([128, NC, D], F32, tag="ch")
                    nc.sync.dma_start(lch[:], lrow_bn[:, n0:n0 + NC, :])
                    pr3 = prodp.tile([128, NC, D], F32, tag="pr")
                    eng(c).tensor_tensor(pr3[:], lch[:], gbb, op=OP.mult)
                    nc.vector.tensor_reduce(
                        logits[:, n0:n0 + NC], pr3[:], axis=AX.X, op=OP.add)

                # ---- tanh clip, mask, store (host does -logsumexp) ----
                tnh = work.tile([128, N], F32, tag="tnh")
                nc.scalar.activation(tnh[:], logits[:], ACTF.Tanh)
                lm = work.tile([128, N], F32, tag="lm")
                nc.vector.tensor_scalar(lm[:], tnh[:], 10.0, None, op0=OP.mult)
                nc.vector.tensor_tensor(lm[:], lm[:], amask[:], op=OP.add)
                nc.sync.dma_start(out[:, bass.ds(t * N, N)], lm[:])

                # ---- argmax + state update ----
                mx8 = small.tile([128, 8], F32, tag="mx8")
                nc.vector.max(mx8[:], lm[:])
                ix8 = small.tile([128, 8], dt.uint32, tag="ix8")
                nc.vector.max_index(ix8[:], mx8[:], lm[:])
                nc.vector.tensor_copy(sel[:], ix8[:, 0:1])
                ohi = small.tile([128, N], dt.int32, tag="ohi")
                nc.vector.tensor_tensor(ohi[:], iota_n[:], sel[:].broadcast_to([128, N]), op=OP.is_equal)
                ohf = small.tile([128, N], F32, tag="ohf")
                nc.vector.tensor_copy(ohf[:], ohi[:])
                nc.vector.scalar_tensor_tensor(
                    amask[:], ohf[:], NEG, amask[:], op0=OP.mult, op1=OP.add)

            if DBG:
                step_body(0)
                nc.sync.dma_start(dbg_fixed2[:, :], fixed2[:])
                nc.sync.dma_start(dbg_q[:, :], qf[:])
                nc.sync.dma_start(dbg_compat[:, :], compat[:].rearrange("p n h -> p (n h)"))
                nc.vector.tensor_copy(compat[:], attn[:])
                nc.sync.dma_start(dbg_attn[:, :], compat[:].rearrange("p n h -> p (n h)"))
                nc.vector.tensor_copy(qf[:], gacc[:])
                nc.sync.dma_start(dbg_g[:, :], qf[:])
                nc.sync.dma_start(dbg_logits[:, :], logits[:])
                nc.sync.dma_start(dbg_sel[:, :], sel[:])
                kch0 = stream.tile([128, NC, D], F32, tag="ch")
                nc.sync.dma_start(kch0[:], krow_bn[:, 20:40, :])
                nc.vector.tensor_copy(qf[:], kch0[:, 10, :])
                nc.sync.dma_start(dbg_krow[:, :], qf[:])
                lch0 = stream.tile([128, NC, D], F32, tag="ch")
                nc.sync.dma_start(lch0[:], lrow_bn[:, 20:40, :])
                nc.vector.tensor_copy(qf[:], lch0[:, 10, :])
                nc.sync.dma_start(dbg_lrow[:, :], qf[:])
                sr0 = small.tile([128, D], F32, tag="sgath")
                nc.sync.dma_start(sr0[:], s_d.rearrange("(b n) c -> b n c", b=BS)[:, 24, :])
                nc.sync.dma_start(dbg_srow[:, :], sr0[:])
            else:
                tc.For_i_unrolled(0, T, 1, step_body, max_unroll=2)

    nc.compile()
    return nc


_CACHE = {}


def kernel(**inputs) -> np.ndarray:
    if "nc" not in _CACHE:
        _CACHE["nc"] = _build()
    nc = _CACHE["nc"]

    emb = np.ascontiguousarray(np.asarray(inputs["embeddings"], np.float32))
    shared = {
        "pref_embed": np.asarray(inputs["pref_embed"], np.float32),
        "W_node": np.asarray(inputs["W_node"], np.float32),
        "W_fixed": np.asarray(inputs["W_fixed"], np.float32),
        "W_step": np.asarray(inputs["W_step"], np.float32),
        "W_out": np.asarray(inputs["W_out"], np.float32),
    }
    in_maps = []
    for i in range(NCORES):
        m = {"embeddings": emb[i * BS:(i + 1) * BS]}
        m.update(shared)
        in_maps.append(m)

    res = run_bass_kernel_spmd(nc, in_maps, list(range(NCORES)))
    outs = [res.results[i]["log_p"].reshape(BS, T, N) for i in range(NCORES)]
    lm = np.concatenate(outs, axis=0)  # (B, T, N): 10*tanh + mask, pre-normalization
    # host-side log_softmax normalization (exact, float64)
    x = lm.astype(np.float64)
    xf = np.where(x > -1e8, x, -np.inf)
    mx = xf.max(axis=2, keepdims=True)
    lse = mx + np.log(np.exp(xf - mx).sum(axis=2, keepdims=True))
    return (x - lse).astype(np.float32)


if __name__ == "__main__":
    z = np.load("inputs.npz")
    inp = {k: z[k] for k in z.files}
    o = kernel(**inp)
    print("kernel output", o.shape, o.dtype)
    np.save("kernel_out.npy", o)
